# revision 38
# baseline (speedup 1.0000x reference)
"""Trainium2 Bass kernel for a dense pre-LN transformer block.

Sharding (8 NeuronCores):
  - 2 groups of 4 cores; group g handles batch g.
  - Within a group, attention is head-parallel: core owns 4 of 16 heads over
    the full 2048-token batch.
  - After attention, two 8-way AllToAlls (one per head pair) re-shard the
    attention output from head-parallel to token-parallel (each core ends
    with all 1024 o-features for its own 512 tokens).  Shards destined to
    the other batch group carry duplicate data and are neutralized by zero
    rows in a per-core zero-padded Wproj (the program is rank-invariant;
    only input data differs).  The first AllToAll overlaps the second head
    pair's compute; the proj contraction is staged so the first half starts
    while the second AllToAll is in flight.
  - proj, LN2 and the MLP are token-parallel: core computes its own 512 rows.

Matmul inputs are float32r (tf32); accumulation, layernorm, softmax
normalization and the residual stream stay fp32.  The softmax needs no
max-subtraction (scores are O(1)); the denominator comes from a ones-column
appended to V inside the same PE accumulation.  LayerNorm gains/biases are
folded into the adjacent weights on the host.
"""

import os
import sys

if "/opt/trn_rl_repo" not in sys.path:
    sys.path.insert(0, "/opt/trn_rl_repo")

import numpy as np

import concourse.bass as bass
import concourse.mybir as mybir
from concourse import bacc, tile
from concourse import bass_utils

B, S, D, H = 2, 2048, 1024, 16
HS = D // H            # 64
EPS = 1e-5
N_CORES = 8
GROUP = 4              # cores per batch group
HPC = 4                # heads per core
TOK = S // GROUP       # own tokens per core (512)

F32 = mybir.dt.float32
F32R = mybir.dt.float32r
RDT = F32 if os.environ.get("KF32") else F32R
AF = mybir.ActivationFunctionType
ALU = mybir.AluOpType

_CACHE = {}

_PHASES = ["A", "B", "C0", "C", "D", "E", "F"]


def _build(phase="F"):
    nc = bacc.Bacc("TRN2", target_bir_lowering=False, debug=False,
                   enable_asserts=True, num_devices=N_CORES)

    tens = {}
    tens["xb"] = nc.dram_tensor("xb", [S, D], F32, kind="ExternalInput").ap()
    # QKV weights with LN1 gain folded; wq also carries the 1/sqrt(HS) scale.
    tens["wq"] = nc.dram_tensor("wq", [128, 8, 2, 128], RDT, kind="ExternalInput").ap()
    tens["wk"] = nc.dram_tensor("wk", [128, 8, 2, 128], RDT, kind="ExternalInput").ap()
    tens["wv"] = nc.dram_tensor("wv", [128, 8, 256], RDT, kind="ExternalInput").ap()
    tens["bqk"] = nc.dram_tensor("bqk", [128, 2, 2], F32, kind="ExternalInput").ap()
    tens["bv"] = nc.dram_tensor("bv", [1, 256], RDT, kind="ExternalInput").ap()
    tens["wpd"] = nc.dram_tensor("wpd", [16, 128, D], RDT, kind="ExternalInput").ap()
    tens["w1"] = nc.dram_tensor("w1", [32, 128, 8, 128], RDT, kind="ExternalInput").ap()
    tens["w2"] = nc.dram_tensor("w2", [2, 32, 128, 512], RDT, kind="ExternalInput").ap()
    tens["xpb"] = nc.dram_tensor("xpb", [4, 128, D], F32, kind="ExternalInput").ap()
    tens["b1t"] = nc.dram_tensor("b1t", [128, 32], F32, kind="ExternalInput").ap()
    tens["b2b"] = nc.dram_tensor("b2b", [128, D], F32, kind="ExternalInput").ap()
    tens["ident"] = nc.dram_tensor("ident", [128, 128], F32, kind="ExternalInput").ap()
    tens["onesc"] = nc.dram_tensor("onesc", [128, 128], RDT, kind="ExternalInput").ap()
    tens["y"] = nc.dram_tensor("y", [TOK, D], F32, kind="ExternalOutput").ap()

    dbg = {}
    if phase != "F":
        shapes = {
            "A": {"h1T": [128, 8 * S]},
            "B": {"QT": [128, 2 * S], "KT": [128, 2 * S], "Vt": [128, 16 * 4 * 65]},
            "C0": {"dbin_a": [1024, 512], "dbin_b": [1024, 512]},
            "C": {"dbout_a": [1024, 512], "dbout_b": [1024, 512]},
            "D": {"x2": [128, 4 * D]},
            "E": {"h2T": [128, 8 * TOK]},
        }[phase]
        for k, shp in shapes.items():
            dbg[k] = nc.dram_tensor(f"dbg_{k}", shp, F32,
                                    kind="ExternalOutput").ap()

    with tile.TileContext(nc) as tc:
        with nc.allow_low_precision(reason="tf32 matmul inputs by design"):
            _emit(nc, tc, tens, phase, dbg)
    nc.compile()
    return nc


def _ln_tile(nc, lns, src_ap, tag, eps_s):
    """LayerNorm stats for one [128, 1024] tile -> (mean, rstd) [128,1]."""
    stats = lns.tile([128, 2, 6], F32, tag=f"{tag}stats")
    nc.vector.bn_stats(stats[:, 0, :], src_ap[:, 0:512])
    nc.vector.bn_stats(stats[:, 1, :], src_ap[:, 512:1024])
    mv = lns.tile([128, 2], F32, tag=f"{tag}mv")
    nc.vector.bn_aggr(mv[:], stats[:])
    std = lns.tile([128, 1], F32, tag=f"{tag}std")
    nc.scalar.activation(std[:], mv[:, 1:2], AF.Sqrt, bias=eps_s[:])
    r1 = lns.tile([128, 1], F32, tag=f"{tag}r1")
    nc.vector.reciprocal(r1[:], std[:])
    return mv[:, 0:1], r1


def _ln_transpose(nc, ps2, lnp, lns, src_ap, dstT, st, idt, tag, eps_s):
    """One [128,1024] tile: LN stats + prenorm + PE transpose into dstT."""
    mean, rstd = _ln_tile(nc, lns, src_ap, tag, eps_s)
    pn = lnp.tile([128, D], F32, tag=f"{tag}pn")
    nc.vector.tensor_scalar(pn[:], src_ap, mean, rstd[:],
                            ALU.subtract, ALU.mult)
    pst = ps2.tile([128, 1024], F32, tag="ps2")
    for dt in range(8):
        nc.tensor.transpose(pst[:, 128 * dt:128 * (dt + 1)],
                            pn[:, 128 * dt:128 * (dt + 1)], idt[:])
    nc.scalar.copy(
        dstT[:, :, 128 * st:128 * (st + 1)],
        pst[:].rearrange("p (dt t) -> p dt t", dt=8))


def _emit(nc, tc, tens, phase, dbg):
    xb, wpd, w1, w2, xpb, y = (tens["xb"], tens["wpd"], tens["w1"],
                               tens["w2"], tens["xpb"], tens["y"])

    with tc.tile_pool(name="const", bufs=1) as const, \
         tc.tile_pool(name="ps2", bufs=2, space="PSUM") as ps2, \
         tc.tile_pool(name="ps_av", bufs=4, space="PSUM") as ps_av, \
         tc.tile_pool(name="dram", bufs=1, space="DRAM") as dram, \
         tc.tile_pool(name="wst", bufs=12) as wst:

        # ---------- constants; big weight tiles load after the first
        # ---------- x tiles (the emission point sets sync-queue order)
        idt = const.tile([128, 128], F32)
        nc.sync.dma_start(idt[:], tens["ident"][:])
        eps_s = const.tile([128, 1], F32, tag="eps")
        nc.vector.memset(eps_s[:], EPS)
        b1t_s = const.tile([128, 32], F32, tag="b1t")
        b2b_s = const.tile([128, D], F32, tag="b2b")
        onesc_s = const.tile([128, 128], RDT, tag="onesc")
        ones64 = onesc_s[0:1, 0:64]
        onestok = onesc_s[0:1, :]
        bqk_s = const.tile([128, 2, 2], F32, tag="bqk")
        bv_s = const.tile([1, 256], RDT, tag="bv")
        wq_s = const.tile([128, 8, 2, 128], RDT, tag="wq")
        wk_s = const.tile([128, 8, 2, 128], RDT, tag="wk")
        wv_s = const.tile([128, 8, 256], RDT, tag="wv")

        def _load_big_consts():
            nc.sync.dma_start(wq_s[:], tens["wq"][:])
            nc.sync.dma_start(wk_s[:], tens["wk"][:])
            nc.sync.dma_start(wv_s[:], tens["wv"][:])
            nc.sync.dma_start(bqk_s[:], tens["bqk"][:])
            nc.sync.dma_start(bv_s[:], tens["bv"][:])
            nc.sync.dma_start(onesc_s[:], tens["onesc"][:])
            nc.sync.dma_start(b1t_s[:], tens["b1t"][:])
            nc.sync.dma_start(b2b_s[:], tens["b2b"][:])

        # PE warm-up: throwaway matmuls so HAM is at full clock by the
        # time the first transposes arrive.
        for wu in range(24):
            wps = ps2.tile([128, 1024], F32, tag="ps2")
            nc.tensor.matmul(wps[:, 0:128], idt[:], idt[:],
                             start=True, stop=True)

        bin_a = dram.tile([1024, 512], RDT, tag="bin_a", name="bin_a")
        bout_a = dram.tile([1024, 512], RDT, tag="bout_a", name="bout_a")
        bin_b = dram.tile([1024, 512], RDT, tag="bin_b", name="bin_b")
        bout_b = dram.tile([1024, 512], RDT, tag="bout_b", name="bout_b")

        with tc.tile_pool(name="x2p", bufs=1) as x2_pool:
            x2 = x2_pool.tile([128, 4, D], F32, tag="x2")

            with tc.tile_pool(name="pina", bufs=1) as pina_pool:
                pin_a = pina_pool.tile([128, 8, 512], RDT, tag="pina")
                wpd_t = {}

                def _load_wpd_half(dc):
                    for et in range(16):
                        wt = wst.tile([128, 1024], RDT, tag="w",
                                      name=f"wpd{dc}_{et}")
                        nc.sync.dma_start(
                            wt[:, 0:512], wpd[et, :, 512 * dc:512 * (dc + 1)])
                        wpd_t[(dc, et)] = wt

                # ================== attention half ==================
                with tc.tile_pool(name="qkv", bufs=1) as qkv_pool:
                    QT = qkv_pool.tile([128, 2, S], RDT, tag="QT")
                    KT = qkv_pool.tile([128, 2, S], RDT, tag="KT")
                    Vt = qkv_pool.tile([128, 16, 4, 65], RDT, tag="Vt")
                    nc.sync.dma_start(
                        Vt[:, :, :, 64],
                        tens["onesc"][:, 0:64].rearrange(
                            "p (a b) -> p a b", a=16))

                    # Phases A+B interleaved: LN1 -> h1T chunk; QKV per
                    # 512-token chunk.  h1T chunk: [128 (d in), 8 (d out),
                    # 512]; QT/KT: [128, 2, 2048] (tile pt = heads
                    # {2pt,2pt+1}); V: [128, 16, 4, 65]
                    with tc.tile_pool(name="h1", bufs=2) as h1_pool, \
                         tc.tile_pool(name="ln1", bufs=2) as lnp, \
                         tc.tile_pool(name="ln1s", bufs=4) as lns:
                        for tc4 in range(4):
                            h1T = h1_pool.tile([128, 8, 512], RDT, tag="h1T",
                                               name=f"h1T{tc4}")
                            for st_loc in range(4):
                                st = 4 * tc4 + st_loc
                                xt = lnp.tile([128, D], F32, tag="xt")
                                nc.sync.dma_start(
                                    xt[:], xb[128 * st:128 * (st + 1), :])
                                _ln_transpose(nc, ps2, lnp, lns, xt[:],
                                              h1T, st_loc, idt, "a", eps_s)
                            if phase == "A":
                                nc.gpsimd.dma_start(
                                    dbg["h1T"].rearrange(
                                        "p (a b) -> p a b",
                                        a=8)[:, :, 512 * tc4:512 * (tc4 + 1)],
                                    h1T[:])
                                if tc4 == 3:
                                    return
                            # ---- QKV for this 512-token chunk ----
                            if tc4 == 0:
                                _load_big_consts()
                                _load_wpd_half(0)
                            tsl = slice(512 * tc4, 512 * (tc4 + 1))
                            for wten, dst, col in ((wq_s, QT, 0),
                                                   (wk_s, KT, 1)):
                                for pt in range(2):
                                    acc = ps2.tile([128, 1024], F32,
                                                   tag="ps2")
                                    for dt in range(8):
                                        nc.tensor.matmul(
                                            acc[:, 0:512],
                                            wten[:, dt, pt, :],
                                            h1T[:, dt, :],
                                            start=(dt == 0), stop=(dt == 7))
                                    nc.vector.tensor_scalar_add(
                                        dst[:, pt, tsl], acc[:, 0:512],
                                        bqk_s[:, pt, col:col + 1])
                            for tcl in range(4):
                                tc16 = 4 * tc4 + tcl
                                acc = ps2.tile([128, 1024], F32, tag="ps2")
                                csl = slice(128 * tcl, 128 * (tcl + 1))
                                for dt in range(8):
                                    nc.tensor.matmul(
                                        acc[:, 0:256], h1T[:, dt, csl],
                                        wv_s[:, dt, :],
                                        start=(dt == 0), stop=False)
                                nc.tensor.matmul(acc[:, 0:256], onestok,
                                                 bv_s[:], start=False,
                                                 stop=True)
                                nc.scalar.copy(
                                    Vt[:, tc16, :, 0:64],
                                    acc[:, 0:256].rearrange(
                                        "p (h e) -> p h e", h=4))

                    if phase == "B":
                        nc.gpsimd.dma_start(
                            dbg["QT"].rearrange("p (a b) -> p a b", a=2),
                            QT[:])
                        nc.gpsimd.dma_start(
                            dbg["KT"].rearrange("p (a b) -> p a b", a=2),
                            KT[:])
                        nc.gpsimd.dma_start(
                            dbg["Vt"], Vt[:].rearrange("p a b c -> p (a b c)"))
                        return

                    # Phase C: scores + exp + AV per head pair
                    with tc.tile_pool(name="et", bufs=7) as etp, \
                         tc.tile_pool(name="att", bufs=3) as att:
                        for hp in range(2):
                            for sb in range(4):
                                pav = [ps_av.tile([65, 512], F32, tag="pav",
                                                  name=f"pav{hp}_{sb}_{i}")
                                       for i in range(2)]
                                for tcc in range(16):
                                    psc = ps2.tile([128, 1024], F32,
                                                   tag="ps2")
                                    for hq in range(2):
                                        nc.tensor.matmul(
                                            psc[:, 512 * hq:512 * (hq + 1)],
                                            KT[64 * hq:64 * (hq + 1), hp,
                                               128 * tcc:128 * (tcc + 1)],
                                            QT[64 * hq:64 * (hq + 1), hp,
                                               512 * sb:512 * (sb + 1)],
                                            start=True, stop=True)
                                    et = etp.tile([128, 1024], RDT, tag="et")
                                    nc.scalar.activation(et[:], psc[:],
                                                         AF.Exp)
                                    for hq in range(2):
                                        nc.tensor.matmul(
                                            pav[hq][:],
                                            Vt[:, tcc, 2 * hp + hq, :],
                                            et[:, 512 * hq:512 * (hq + 1)],
                                            start=(tcc == 0),
                                            stop=(tcc == 15))
                                for hq in range(2):
                                    rec = att.tile([1, 512], RDT, tag="rec")
                                    nc.vector.reciprocal(rec[:],
                                                         pav[hq][64:65, :])
                                    pbct = ps_av.tile(
                                        [65, 512], F32, tag="pav",
                                        name=f"pbc{hp}_{sb}_{hq}")
                                    pbc = pbct[0:64, :]
                                    nc.tensor.matmul(pbc, ones64, rec[:],
                                                     start=True, stop=True)
                                    rb = att.tile([64, 512], F32, tag="rb")
                                    nc.vector.tensor_copy(rb[:], pbc)
                                    ot = att.tile([64, 512], RDT, tag="ot")
                                    nc.vector.tensor_mul(
                                        ot[:], pav[hq][0:64, :], rb[:])
                                    # both group slots (finite wire data)
                                    bin_hp = bin_a if hp == 0 else bin_b
                                    nc.gpsimd.dma_start(
                                        bin_hp[128 * sb + 64 * hq:
                                               128 * sb + 64 * (hq + 1), :],
                                        ot[:])
                                    nc.gpsimd.dma_start(
                                        bin_hp[128 * (sb + 4) + 64 * hq:
                                               128 * (sb + 4) +
                                               64 * (hq + 1), :],
                                        ot[:])
                            # fire this head-pair's A2A; the first one
                            # overlaps the second head pair's compute
                            nc.gpsimd.collective_compute(
                                "AllToAll", ALU.bypass,
                                replica_groups=[list(range(N_CORES))],
                                ins=[(bin_a if hp == 0 else bin_b).opt()],
                                outs=[(bout_a if hp == 0 else bout_b).opt()])
                            if hp == 0 and phase not in ("C0",):
                                # heads {0,1} arrive now; stage them for proj
                                for hh in range(2):
                                    for q in range(4):
                                        for half in range(2):
                                            src = (128 * (2 * q + half) +
                                                   64 * hh)
                                            nc.gpsimd.dma_start(
                                                pin_a[64 * half:
                                                      64 * (half + 1),
                                                      4 * hh + q, :],
                                                bout_a[src:src + 64, :])
                    if phase == "C0":
                        nc.gpsimd.dma_start(dbg["dbin_a"], bin_a[:])
                        nc.gpsimd.dma_start(dbg["dbin_b"], bin_b[:])
                        return
                    if phase == "C":
                        nc.gpsimd.dma_start(dbg["dbout_a"], bout_a[:])
                        nc.gpsimd.dma_start(dbg["dbout_b"], bout_b[:])
                        return

                # ============== Phase D: proj + residual -> x2 ==============
                with tc.tile_pool(name="proj", bufs=1) as prj:
                    pin_b = prj.tile([128, 8, 512], RDT, tag="pinb")
                    for hh in range(2):
                        for q in range(4):
                            for half in range(2):
                                src = 128 * (2 * q + half) + 64 * hh
                                nc.gpsimd.dma_start(
                                    pin_b[64 * half:64 * (half + 1),
                                          4 * hh + q, :],
                                    bout_b[src:src + 64, :])
                    xpb_s = prj.tile([128, 4, D], F32, tag="xpb")
                    nc.sync.dma_start(xpb_s[:],
                                      xpb.rearrange("st p d -> p st d"))
                    for dc in range(2):
                        if dc == 1:
                            _load_wpd_half(1)
                        accs = [ps2.tile([128, 1024], F32, tag="ps2",
                                         name=f"acp{dc}_{i}")
                                for i in range(2)]
                        # ets 0-7 (heads {0,1}) are available before the
                        # second A2A completes; 8-15 follow.
                        for et in range(16):
                            pin_src = pin_a if et < 8 else pin_b
                            for sc in range(4):
                                nc.tensor.matmul(
                                    accs[sc // 2][:, 512 * (sc % 2):
                                                  512 * (sc % 2 + 1)],
                                    pin_src[:, et % 8,
                                            128 * sc:128 * (sc + 1)],
                                    wpd_t[(dc, et)][:, 0:512],
                                    start=(et == 0), stop=(et == 15))
                        for sc in range(4):
                            nc.vector.tensor_add(
                                x2[:, sc, 512 * dc:512 * (dc + 1)],
                                accs[sc // 2][:, 512 * (sc % 2):
                                              512 * (sc % 2 + 1)],
                                xpb_s[:, sc, 512 * dc:512 * (dc + 1)])

            if phase == "D":
                nc.gpsimd.dma_start(
                    dbg["x2"].rearrange("p (a b) -> p a b", a=4), x2[:])
                return

            # Phase E/F: LN2 -> h2T ; MLP ; output (LN2 g/b folded into W1/b1)
            with tc.tile_pool(name="mt", bufs=1) as mt_pool:
                mT = mt_pool.tile([128, 32, TOK], RDT, tag="mT")

                with tc.tile_pool(name="h2", bufs=1) as h2_pool:
                    h2T = h2_pool.tile([128, 8, TOK], RDT, tag="h2T")
                    with tc.tile_pool(name="ln2", bufs=2) as lnp2, \
                         tc.tile_pool(name="ln2s", bufs=4) as lns2:
                        for st in range(4):
                            _ln_transpose(nc, ps2, lnp2, lns2,
                                          x2[:, st, :], h2T, st, idt, "b",
                                          eps_s)
                    if phase == "E":
                        nc.gpsimd.dma_start(
                            dbg["h2T"].rearrange("p (a b) -> p a b", a=8),
                            h2T[:])
                        return

                    # MLP up: mT = relu(W1'^T h2 + b1')
                    for fc in range(32):
                        wt = wst.tile([128, 1024], RDT, tag="w",
                                      name=f"w1_{fc}")
                        wtv = wt[:].rearrange("p (dt f) -> p dt f", dt=8)
                        nc.sync.dma_start(
                            wtv, w1[fc])
                        acc = ps2.tile([128, 1024], F32, tag="ps2")
                        for dt in range(8):
                            nc.tensor.matmul(
                                acc[:, 0:512], wtv[:, dt, :], h2T[:, dt, :],
                                start=(dt == 0), stop=(dt == 7))
                        nc.scalar.activation(mT[:, fc, :], acc[:, 0:512],
                                             AF.Relu,
                                             bias=b1t_s[:, fc:fc + 1])

                # MLP down + residual + b2 -> y
                with tc.tile_pool(name="yout", bufs=2) as yp:
                    for dc in range(2):
                        accs = [ps2.tile([128, 1024], F32, tag="ps2",
                                         name=f"acy{dc}_{i}")
                                for i in range(2)]
                        for ft in range(32):
                            wt = wst.tile([128, 1024], RDT, tag="w",
                                          name=f"w2_{dc}_{ft}")
                            nc.sync.dma_start(wt[:, 0:512], w2[dc, ft])
                            for sc in range(4):
                                nc.tensor.matmul(
                                    accs[sc // 2][:, 512 * (sc % 2):
                                                  512 * (sc % 2 + 1)],
                                    mT[:, ft, 128 * sc:128 * (sc + 1)],
                                    wt[:, 0:512],
                                    start=(ft == 0), stop=(ft == 31))
                        for sc in range(4):
                            yt = yp.tile([128, 512], F32, tag="yt")
                            nc.vector.tensor_add(
                                yt[:],
                                accs[sc // 2][:, 512 * (sc % 2):
                                              512 * (sc % 2 + 1)],
                                x2[:, sc, 512 * dc:512 * (dc + 1)])
                            nc.vector.tensor_add(
                                yt[:], yt[:],
                                b2b_s[:, 512 * dc:512 * (dc + 1)])
                            nc.sync.dma_start(
                                y[128 * sc:128 * (sc + 1),
                                  512 * dc:512 * (dc + 1)],
                                yt[:])


def _get_nc(phase="F"):
    key = f"nc_{phase}"
    if key not in _CACHE:
        _CACHE[key] = _build(phase)
    return _CACHE[key]


def _tf32_round(a):
    """Round fp32 to tf32 (10-bit mantissa) with round-to-nearest-even."""
    if RDT is F32:
        return np.ascontiguousarray(a)
    a = np.ascontiguousarray(a)
    b = a.view(np.uint32)
    lsb = (b >> np.uint32(13)) & np.uint32(1)
    out = (b + np.uint32(0xFFF) + lsb) & np.uint32(0xFFFFE000)
    return out.view(np.float32)


def _prep_in_maps(x, Wq, Wk, Wv, Wproj, bproj, ln1_g, ln1_b, ln2_g, ln2_b,
                  W1, b1, W2, b2):
    f = np.float32
    x = np.asarray(x, f)
    Wq = np.asarray(Wq, f); Wk = np.asarray(Wk, f); Wv = np.asarray(Wv, f)
    Wproj = np.asarray(Wproj, f); bproj = np.asarray(bproj, f)
    ln1_g = np.asarray(ln1_g, f); ln1_b = np.asarray(ln1_b, f)
    ln2_g = np.asarray(ln2_g, f); ln2_b = np.asarray(ln2_b, f)
    W1 = np.asarray(W1, f); b1 = np.asarray(b1, f)
    W2 = np.asarray(W2, f); b2 = np.asarray(b2, f)

    scale = f(HS) ** f(-0.5)
    ident = np.eye(128, dtype=f)
    onesv = np.ones((128, 128), dtype=f)
    # LN2 gain folded into W1 rows; LN2 bias folded into b1.
    W1g = W1 * ln2_g[:, None]
    b1f = b1 + ln2_b @ W1
    b1tv = np.ascontiguousarray(b1f.reshape(32, 128).T)
    b2bv = np.ascontiguousarray(np.tile(b2[None, :], (128, 1)))
    w1v = _tf32_round(W1g.reshape(8, 128, 32, 128).transpose(2, 1, 0, 3))
    w2v = _tf32_round(W2.reshape(32, 128, 2, 512).transpose(2, 0, 1, 3))

    in_maps = []
    for c in range(N_CORES):
        g, r = divmod(c, GROUP)
        heads = [4 * r + hh for hh in range(HPC)]
        # LN1 gain folded into QKV weight rows; LN1 bias -> bqk / bv.
        wq_c = np.concatenate([Wq[hh] for hh in heads], axis=1) * scale
        wk_c = np.concatenate([Wk[hh] for hh in heads], axis=1)
        wv_c = np.concatenate([Wv[hh] for hh in heads], axis=1)
        bq_c = ln1_b @ wq_c     # [256]
        bk_c = ln1_b @ wk_c
        bv_c = ln1_b @ wv_c
        wq_g = wq_c * ln1_g[:, None]
        wk_g = wk_c * ln1_g[:, None]
        wv_g = wv_c * ln1_g[:, None]
        wq_r = _tf32_round(wq_g.reshape(8, 128, 2, 128).transpose(1, 0, 2, 3))
        wk_r = _tf32_round(wk_g.reshape(8, 128, 2, 128).transpose(1, 0, 2, 3))
        wv_r = _tf32_round(wv_g.reshape(8, 128, 256).transpose(1, 0, 2))
        bqk_v = np.stack([bq_c.reshape(2, 128).T, bk_c.reshape(2, 128).T],
                         axis=2)  # [128, 2(pt), 2(q/k)]
        bv_v = _tf32_round(bv_c.reshape(1, 256))
        # zero-padded proj: e-tile (4*h_slot + i//2), rows 64*(i%2)
        wpd_v = np.zeros((16, 128, D), dtype=f)
        for h_slot in range(HPC):
            for i in range(N_CORES):
                if i // GROUP == g:
                    gh = 4 * (i % GROUP) + h_slot
                    blk = 64 * (i % 2)
                    wpd_v[4 * h_slot + i // 2, blk:blk + 64, :] = \
                        Wproj[64 * gh:64 * (gh + 1), :]
        wpd_v = _tf32_round(wpd_v)
        xrows = x[g, TOK * r:TOK * (r + 1), :] + bproj
        in_maps.append({
            "xb": np.ascontiguousarray(x[g]),
            "wq": wq_r, "wk": wk_r, "wv": wv_r,
            "bqk": np.ascontiguousarray(bqk_v), "bv": bv_v,
            "wpd": wpd_v, "w1": w1v, "w2": w2v,
            "xpb": np.ascontiguousarray(xrows.reshape(4, 128, D)),
            "b1t": b1tv, "b2b": b2bv, "ident": ident, "onesc": onesv,
        })
    return in_maps


def run(inputs, trace=False, phase="F"):
    nc = _get_nc(phase)
    in_maps = _prep_in_maps(**inputs)
    res = bass_utils.run_bass_kernel_spmd(
        nc, in_maps, core_ids=list(range(N_CORES)), trace=trace)
    if phase != "F":
        return res.results, res.exec_time_ns
    out = np.empty((B, S, D), dtype=np.float32)
    for c in range(N_CORES):
        g, r = divmod(c, GROUP)
        out[g, TOK * r:TOK * (r + 1), :] = res.results[c]["y"]
    return out, res.exec_time_ns


def kernel(**inputs):
    out, _ = run(inputs)
    return out
